# revision 4
# baseline (speedup 1.0000x reference)
"""Fused attention (RoPE + KV-cache scatter + causal GQA + out-proj) on 8 trn2 cores.

Reference semantics (B=2, S=2048, H=4096, NQ=32, NKV=8, D=128):
    xq = (x @ wq.T)           -> RoPE
    xk = (x @ w_kv[:1024].T)  -> RoPE
    xv = (x @ w_kv[1024:].T)
    new_kv_buffer = kv_buffer.at[select_index].set(concat([xk, xv], head-axis))
    out = softmax(causal(xq xk^T / sqrt(D))) @ xv @ wo.T

Sharding: 8 cores = (2 batches) x (4 q-head groups of 8 heads / 2 kv heads).
Each core computes its batch+head-group slice end-to-end; wo is column-sharded
so each core emits a partial out[2048, 4096] summed on the host.

Per-core dataflow (all matmuls fp32r = full PE rate, ~1e-4 matmul error):
  1. Projections with contraction dim (H) on partitions: host supplies
     pre-transposed xT/wqT/wkT/wvT, producing QT/KT [d, t] and V [t, d].
     RoPE applied via a half-swap permutation matmul + DVE muls with
     host-transposed cos/sin tables. QT spilled to DRAM scratch; KT resident.
  2. Causal attention per head in transposed layout: S.T tiles [j, i] =
     KT_tile^T @ QT, masked+exp'd on ACT into E.T (fp32r), row-sums via
     ones-matmul, O.T accumulation via V-stationary matmuls, normalization
     by reciprocal row sums broadcast across partitions.
  3. Out-projection: out[t, o] = sum_f O.T[f, t] * woT[f, o], per-core
     partial summed on host.
"""

import numpy as np

B, S, H = 2, 2048, 4096
NQ, NKV, D = 32, 8, 128
BUF = 8192
NCORES = 8
G = 4            # head-groups per batch
HPC = NQ // G    # 8 q-heads per core
KVPC = NKV // G  # 2 kv-heads per core
F = HPC * D      # 1024 attn features per core
ISQ = float(D) ** -0.5
NEG = -1.0e9

_CACHE = {}


def _build_nc(repeat=1):
    from contextlib import ExitStack
    import concourse.bacc as bacc
    import concourse.mybir as mybir
    import concourse.tile as tile

    f32 = mybir.dt.float32
    f32r = mybir.dt.float32r
    EXP = mybir.ActivationFunctionType.Exp

    nc = bacc.Bacc("TRN2", target_bir_lowering=False, debug=False)

    xT = nc.dram_tensor("xT", [H, S], f32r, kind="ExternalInput")
    wqT = nc.dram_tensor("wqT", [H, F], f32r, kind="ExternalInput")
    wkT = nc.dram_tensor("wkT", [H, KVPC * D], f32r, kind="ExternalInput")
    wvT = nc.dram_tensor("wvT", [H, KVPC * D], f32r, kind="ExternalInput")
    woT = nc.dram_tensor("woT", [F, H], f32r, kind="ExternalInput")
    cosT = nc.dram_tensor("cosT", [D, S], f32, kind="ExternalInput")
    sinT = nc.dram_tensor("sinT", [D, S], f32, kind="ExternalInput")
    perm = nc.dram_tensor("perm", [D, D], f32r, kind="ExternalInput")
    cmask = nc.dram_tensor("cmask", [D, 384], f32, kind="ExternalInput")
    onesd = nc.dram_tensor("onesd", [D, 1], f32r, kind="ExternalInput")

    outp = nc.dram_tensor("outp", [S, H], f32, kind="ExternalOutput")
    kT_out = nc.dram_tensor("kT_out", [KVPC * D, S], f32r, kind="ExternalOutput")
    v_out = nc.dram_tensor("v_out", [S, KVPC * D], f32r, kind="ExternalOutput")

    qT_scr = nc.dram_tensor("qT_scr", [F, S], f32r)
    v_scr = nc.dram_tensor("v_scr", [S, KVPC * D], f32r)

    NH = H // 128   # 32 h-tiles
    TCH = 1024      # proj t-chunk

    with tile.TileContext(nc) as tc:
        with ExitStack() as octx:
          # K stays SBUF-resident from projection through attention.
          ktp = octx.enter_context(tc.tile_pool(name="ktp", bufs=KVPC + 1))
          for _rep in range(repeat):
            kt_res = [ktp.tile([128, S], f32r, name="ktres", tag="ktres") for _ in range(KVPC)]

            # ---------------- Phase 1: projections + RoPE ----------------
            with ExitStack() as pctx:
                xtp = pctx.enter_context(tc.tile_pool(name="xtp", bufs=NH))
                wst = pctx.enter_context(tc.tile_pool(name="wst", bufs=3))
                wvst = pctx.enter_context(tc.tile_pool(name="wvst", bufs=3))
                trig = pctx.enter_context(tc.tile_pool(name="trig", bufs=1))
                ropep = pctx.enter_context(tc.tile_pool(name="ropep", bufs=2))
                vev = pctx.enter_context(tc.tile_pool(name="vev", bufs=4))

                for tch in range(S // TCH):
                    t0 = tch * TCH
                    xts = []
                    for h in range(NH):
                        xt_t = xtp.tile([128, TCH], f32r, name="xt", tag="xt")
                        nc.sync.dma_start(
                            xt_t[:], xT.ap()[h * 128:(h + 1) * 128, t0:t0 + TCH])
                        xts.append(xt_t)
                    cos_t = trig.tile([128, TCH], f32, name="cos", tag="cos")
                    nc.sync.dma_start(cos_t[:], cosT.ap()[:, t0:t0 + TCH])
                    sin_t = trig.tile([128, TCH], f32, name="sin", tag="sin")
                    nc.sync.dma_start(sin_t[:], sinT.ap()[:, t0:t0 + TCH])
                    perm_t = trig.tile([D, D], f32r, name="perm", tag="perm")
                    nc.sync.dma_start(perm_t[:], perm.ap()[:, :])

                    # V projection: natural [t, d] layout; xT tiles stationary.
                    with ExitStack() as vctx:
                        vps = vctx.enter_context(
                            tc.tile_pool(name="vps", bufs=4, space="PSUM"))
                        for half in range(2):
                            vpsums = [vps.tile([128, KVPC * D], f32, name="vps", tag="vps")
                                      for _ in range(4)]
                            for h in range(NH):
                                wv_t = wvst.tile([128, KVPC * D], f32r, name="wv", tag="wv")
                                nc.sync.dma_start(
                                    wv_t[:], wvT.ap()[h * 128:(h + 1) * 128, :])
                                for tt in range(4):
                                    c0 = (half * 4 + tt) * 128
                                    nc.tensor.matmul(
                                        vpsums[tt][:],
                                        xts[h][:, c0:c0 + 128],
                                        wv_t[:],
                                        start=(h == 0), stop=(h == NH - 1))
                            for tt in range(4):
                                r0 = t0 + (half * 4 + tt) * 128
                                v_t = vev.tile([128, KVPC * D], f32r, name="vt", tag="vt")
                                nc.scalar.copy(v_t[:], vpsums[tt][:])
                                nc.sync.dma_start(
                                    v_scr.ap()[r0:r0 + 128, :], v_t[:])
                                nc.sync.dma_start(
                                    v_out.ap()[r0:r0 + 128, :], v_t[:])

                    # Q (8 slabs) + K (2 slabs): [d, t] layout, rhs = xT tiles.
                    with ExitStack() as qctx:
                        qkp = qctx.enter_context(
                            tc.tile_pool(name="qkp", bufs=4, space="PSUM"))
                        prm = qctx.enter_context(
                            tc.tile_pool(name="prm", bufs=2, space="PSUM"))
                        for slab in range(HPC + KVPC):
                            is_q = slab < HPC
                            wsrc = wqT if is_q else wkT
                            o0 = (slab if is_q else slab - HPC) * 128
                            psA = qkp.tile([128, 512], f32, name="qk", tag="qk")
                            psB = qkp.tile([128, 512], f32, name="qk", tag="qk")
                            for h in range(NH):
                                w_t = wst.tile([128, 128], f32r, name="w", tag="w")
                                nc.sync.dma_start(
                                    w_t[:],
                                    wsrc.ap()[h * 128:(h + 1) * 128, o0:o0 + 128])
                                nc.tensor.matmul(
                                    psA[:], w_t[:], xts[h][:, 0:512],
                                    start=(h == 0), stop=(h == NH - 1))
                                nc.tensor.matmul(
                                    psB[:], w_t[:], xts[h][:, 512:1024],
                                    start=(h == 0), stop=(h == NH - 1))
                            for ts_, ps in enumerate((psA, psB)):
                                c0 = ts_ * 512
                                qsb = ropep.tile([128, 512], f32r, name="qsb", tag="qsb")
                                nc.scalar.copy(qsb[:], ps[:])
                                pps = prm.tile([128, 512], f32, name="pp", tag="pp")
                                nc.tensor.matmul(pps[:], perm_t[:], qsb[:],
                                                 start=True, stop=True)
                                m1 = ropep.tile([128, 512], f32, name="m1", tag="m1")
                                nc.vector.tensor_mul(
                                    m1[:], qsb[:].bitcast(f32),
                                    cos_t[:, c0:c0 + 512])
                                m2 = ropep.tile([128, 512], f32, name="m2", tag="m2")
                                nc.vector.tensor_mul(
                                    m2[:], pps[:], sin_t[:, c0:c0 + 512])
                                if is_q:
                                    dst = ropep.tile([128, 512], f32r, name="ro", tag="ro")
                                    nc.vector.tensor_add(dst[:], m1[:], m2[:])
                                    nc.sync.dma_start(
                                        qT_scr.ap()[o0:o0 + 128,
                                                    t0 + c0:t0 + c0 + 512],
                                        dst[:])
                                else:
                                    kvi = slab - HPC
                                    nc.vector.tensor_add(
                                        kt_res[kvi][:, t0 + c0:t0 + c0 + 512],
                                        m1[:], m2[:])

            for kvi in range(KVPC):
                nc.sync.dma_start(
                    kT_out.ap()[kvi * 128:(kvi + 1) * 128, :], kt_res[kvi][:])

            # ---------------- Phase 2: causal attention ----------------
            ot_tiles = [None] * HPC
            qot = octx.enter_context(tc.tile_pool(name="qot", bufs=HPC + 2))
            with ExitStack() as actx:
                vkv = actx.enter_context(tc.tile_pool(name="vkv", bufs=S // 128))
                etp = actx.enter_context(tc.tile_pool(name="etp", bufs=6))
                cst = actx.enter_context(tc.tile_pool(name="cst", bufs=1))
                sgp = actx.enter_context(tc.tile_pool(name="sgp", bufs=3))
                stp = actx.enter_context(
                    tc.tile_pool(name="stp", bufs=3, space="PSUM"))
                sgps = actx.enter_context(
                    tc.tile_pool(name="sgps", bufs=2, space="PSUM"))
                otp = actx.enter_context(
                    tc.tile_pool(name="otp", bufs=2, space="PSUM"))

                cm = cst.tile([D, 384], f32, name="cm", tag="cm")
                nc.sync.dma_start(cm[:], cmask.ap()[:, :])
                ones_t = cst.tile([D, 1], f32r, name="ones", tag="ones")
                nc.sync.dma_start(ones_t[:], onesd.ap()[:, :])

                for kv in range(KVPC):
                    vts = []
                    for tt in range(S // 128):
                        v_t = vkv.tile([128, 128], f32r, name="vkv", tag="vkv")
                        nc.sync.dma_start(
                            v_t[:],
                            v_scr.ap()[tt * 128:(tt + 1) * 128,
                                       kv * 128:(kv + 1) * 128])
                        vts.append(v_t)
                    for hh in range(HPC // KVPC):
                        h = kv * (HPC // KVPC) + hh
                        qt = qot.tile([128, S], f32r, name="qot", tag="qot")
                        nc.sync.dma_start(
                            qt[:], qT_scr.ap()[h * 128:(h + 1) * 128, :])
                        ot = qot.tile([128, S], f32r, name="qot", tag="qot")
                        ot_tiles[h] = ot
                        for c in range(4):
                            i0 = c * 512
                            ot_ps = otp.tile([128, 512], f32, name="ot", tag="ot")
                            sg_ps = sgps.tile([1, 512], f32, name="sg", tag="sg")
                            njt = 4 * c + 4
                            for jt in range(njt):
                                k = jt - 4 * c
                                if k < 0:
                                    w0, wdt = 0, 512
                                elif k == 0:
                                    w0, wdt = 0, 512
                                elif k == 1:
                                    w0, wdt = 128, 384
                                else:  # k == 2 or 3
                                    w0, wdt = 256, 256
                                st_ps = stp.tile([128, 512], f32, name="st", tag="st")
                                nc.tensor.matmul(
                                    st_ps[:, 0:wdt],
                                    kt_res[kv][:, jt * 128:(jt + 1) * 128],
                                    qt[:, i0 + w0:i0 + w0 + wdt],
                                    start=True, stop=True)
                                et = etp.tile([128, 512], f32r, name="et", tag="et")
                                if k < 0:
                                    nc.scalar.activation(
                                        et[:, 0:512], st_ps[:, 0:512],
                                        EXP, scale=ISQ)
                                elif k < 3:
                                    mt = etp.tile([128, 128], f32, name="mt", tag="mt")
                                    nc.vector.tensor_add(
                                        mt[:], st_ps[:, 0:128], cm[:, 0:128])
                                    nc.scalar.activation(
                                        et[:, 0:128], mt[:], EXP, scale=ISQ)
                                    if wdt > 128:
                                        nc.scalar.activation(
                                            et[:, 128:wdt], st_ps[:, 128:wdt],
                                            EXP, scale=ISQ)
                                else:
                                    mt = etp.tile([128, 256], f32, name="mt2", tag="mt2")
                                    nc.vector.tensor_add(
                                        mt[:], st_ps[:, 0:256], cm[:, 128:384])
                                    nc.scalar.activation(
                                        et[:, 0:256], mt[:], EXP, scale=ISQ)
                                nc.tensor.matmul(
                                    sg_ps[:, w0:w0 + wdt], ones_t[:],
                                    et[:, 0:wdt],
                                    start=(jt == 0), stop=(jt == njt - 1))
                                nc.tensor.matmul(
                                    ot_ps[:, w0:w0 + wdt], vts[jt][:],
                                    et[:, 0:wdt],
                                    start=(jt == 0), stop=(jt == njt - 1))
                            sgs = sgp.tile([1, 512], f32, name="sgs", tag="sgs")
                            nc.scalar.copy(sgs[:], sg_ps[:])
                            rc = sgp.tile([1, 512], f32, name="rc", tag="rc")
                            nc.vector.reciprocal(rc[:], sgs[:])
                            rb = sgp.tile([128, 512], f32, name="rb", tag="rb")
                            nc.gpsimd.partition_broadcast(rb[:], rc[:])
                            nc.vector.tensor_mul(
                                ot[:, i0:i0 + 512], ot_ps[:], rb[:])

            # ---------------- Phase 3: out-projection ----------------
            with ExitStack() as wctx:
                wop = wctx.enter_context(tc.tile_pool(name="wop", bufs=16))
                oev = wctx.enter_context(tc.tile_pool(name="oev", bufs=4))
                opp = wctx.enter_context(
                    tc.tile_pool(name="opp", bufs=2, space="PSUM"))
                for oc in range(H // 512):
                    wts = []
                    for f in range(HPC):
                        w_t = wop.tile([128, 512], f32r, name="wo", tag="wo")
                        nc.sync.dma_start(
                            w_t[:],
                            woT.ap()[f * 128:(f + 1) * 128,
                                     oc * 512:(oc + 1) * 512])
                        wts.append(w_t)
                    for tt in range(S // 128):
                        ps = opp.tile([128, 512], f32, name="op", tag="op")
                        for f in range(HPC):
                            nc.tensor.matmul(
                                ps[:],
                                ot_tiles[f][:, tt * 128:(tt + 1) * 128],
                                wts[f][:],
                                start=(f == 0), stop=(f == HPC - 1))
                        ob = oev.tile([128, 512], f32, name="ob", tag="ob")
                        nc.scalar.copy(ob[:], ps[:])
                        nc.sync.dma_start(
                            outp.ap()[tt * 128:(tt + 1) * 128,
                                      oc * 512:(oc + 1) * 512], ob[:])

    nc.compile()
    return nc


def _host_prep(x, wq, w_kv, wo, cos, sin):
    cosT = np.ascontiguousarray(cos.T.astype(np.float32))
    sinT = cos.T.astype(np.float32).copy()
    sinT[:] = sin.T
    sinT[0:D // 2] *= -1.0
    sinT = np.ascontiguousarray(sinT)
    permM = np.zeros((D, D), np.float32)
    for d in range(D):
        permM[d, (d + D // 2) % D] = 1.0
    # cmask[:, 0:128] = diag mask (0 where j<=i, NEG below); [:, 128:384] =
    # [full NEG | diag] for the k=3 band tile.
    jj = np.arange(128)[:, None]
    ii = np.arange(128)[None, :]
    maskD = np.where(jj <= ii, 0.0, NEG).astype(np.float32)
    cmask = np.concatenate(
        [maskD, np.full((128, 128), NEG, np.float32), maskD], axis=1)
    cmask = np.ascontiguousarray(cmask)
    ones = np.ones((D, 1), np.float32)

    in_maps = []
    for c in range(NCORES):
        b, g = divmod(c, G)
        in_maps.append({
            "xT": np.ascontiguousarray(x[b].T.astype(np.float32)),
            "wqT": np.ascontiguousarray(
            wq[F * g:F * (g + 1), :].T.astype(np.float32)),
            "wkT": np.ascontiguousarray(
            w_kv[KVPC * D * g:KVPC * D * (g + 1), :].T.astype(np.float32)),
            "wvT": np.ascontiguousarray(
            w_kv[NKV * D + KVPC * D * g:
                 NKV * D + KVPC * D * (g + 1), :].T.astype(np.float32)),
            "woT": np.ascontiguousarray(
            wo[:, F * g:F * (g + 1)].T.astype(np.float32)),
            "cosT": cosT,
            "sinT": sinT,
            "perm": permM,
            "cmask": cmask,
            "onesd": ones,
        })
    return in_maps


LAST_RESULT = None


def kernel(x, wq, w_kv, wo, cos, sin, kv_buffer, select_index):
    global LAST_RESULT
    from concourse.bass_utils import run_bass_kernel_spmd

    x = np.asarray(x, dtype=np.float32)
    wq = np.asarray(wq, dtype=np.float32)
    w_kv = np.asarray(w_kv, dtype=np.float32)
    wo = np.asarray(wo, dtype=np.float32)
    cos = np.asarray(cos, dtype=np.float32)
    sin = np.asarray(sin, dtype=np.float32)
    kv_buffer = np.asarray(kv_buffer, dtype=np.float32)
    sel = np.asarray(select_index).astype(np.int64)

    if "nc" not in _CACHE:
        _CACHE["nc"] = _build_nc()
    nc = _CACHE["nc"]

    in_maps = _host_prep(x, wq, w_kv, wo, cos, sin)
    res = run_bass_kernel_spmd(nc, in_maps, list(range(NCORES)))
    LAST_RESULT = res

    out = np.zeros((B, S, H), np.float32)
    comb = np.empty((B * S, 2 * NKV, D), np.float32)
    for c in range(NCORES):
        b, g = divmod(c, G)
        r = res.results[c]
        out[b] += r["outp"]
        kT = r["kT_out"].reshape(KVPC, D, S)
        comb[b * S:(b + 1) * S, KVPC * g:KVPC * (g + 1), :] = (
            kT.transpose(2, 0, 1))
        comb[b * S:(b + 1) * S, NKV + KVPC * g:NKV + KVPC * (g + 1), :] = (
            r["v_out"].reshape(S, KVPC, D))
    new_buf = kv_buffer.copy()
    new_buf[sel] = comb
    return out, new_buf
